# revision 1
# baseline (speedup 1.0000x reference)
"""Trainium2 Bass kernel for an 8-layer GCN (MemoryEfficientGNN).

Strategy (8 NeuronCores, SPMD single program):
  - Nodes sharded across cores: core k owns real nodes [12500k, 12500(k+1)),
    padded to 12544 rows (98 tiles/windows of 128).
  - GCN norm factorizes: out[v] = dinv[v] * sum_{(u,v)} dinv[u]*h[u], so the
    per-edge weights are one-hot.
  - Per layer: h' = dinv*(x_own @ W) on PE (bf16), AllGather h' into a
    [100352,128] bf16 HBM tensor, then scatter-aggregate own dest windows:
    per 128-edge batch, dma_gather source rows and one-hot matmul
    P^T.T @ G accumulated in PSUM per 128-dest window.
  - dma_gather takes int16 indices, so sources are chunked 4x25088 rows;
    batches are grouped (window-block of 4, chunk) so each gather call
    reads one chunk while PSUM holds the block's 4 window accumulators.
  - Post per window: dinv scale + bias + LayerNorm + affine + ReLU
    (+ residual on alternate mid layers) on DVE/ACT.
  - Final: per-graph mean pool via one-hot(batch-id) matmuls, AllReduce of
    [128 graphs, 128+1] partials, replicated MLP + sigmoid.

The batch schedule is static and identical on all cores (SPMD); per-core
data (indices, offsets, x rows) differ.
"""

import sys

sys.path.insert(0, "/opt/trn_rl_repo")

import numpy as np
import ml_dtypes

N_NODES = 100000
N_EDGES = 1600000
HID = 128
FC_DIM = 64
NUM_GRAPHS = 128
EPS = 1e-5
NCORES = 8
OWN = N_NODES // NCORES          # 12500 real nodes per core
NTILES = (OWN + 127) // 128      # 98 windows per core
R = NTILES * 128                 # 12544 padded rows per core
GPAD = R * NCORES                # 100352 padded global rows
P = 128
NCHUNK = 4
CHUNK = GPAD // NCHUNK           # 25088 rows per gather chunk (int16-safe)
WB = 4                           # windows per PSUM block

BF = ml_dtypes.bfloat16

# fp32 constant-blob layout: one DMA covers every fp32 constant so any
# first-reader sees a single DMA semaphore lane (walrus caps per-inst waits).
_CB_LAYOUT = [
    ("W_in", P), ("W_mid", P), ("W_out", P), ("identity", P), ("iota_f32", P),
    ("b_in_bc", P), ("g_in_bc", P), ("be_in_bc", P),
    ("b_mid_bc", P), ("g_mid_bc", P), ("be_mid_bc", P),
    ("b_out_bc", P), ("g_out_bc", P), ("be_out_bc", P),
    ("b1_bc", FC_DIM), ("W1", FC_DIM), ("W2", 1),
    ("epsb", 1), ("ones_col", 1), ("b2_bc", 1),
    ("dinvs", NTILES), ("batchb", NTILES),
]
CB_OFF = {}
_o = 0
for _n, _w in _CB_LAYOUT:
    CB_OFF[_n] = _o
    _o += _w
CB_COLS = _o


def _cb_slices():
    return [(n, CB_OFF[n], w) for n, w in _CB_LAYOUT if n != "W2"]


# ---------------------------------------------------------------- host prep
def build_schedule(edge_index):
    """Static batch schedule + per-core index/offset arrays."""
    row = np.concatenate([edge_index[0], np.arange(N_NODES, dtype=np.int64)])
    col = np.concatenate([edge_index[1], np.arange(N_NODES, dtype=np.int64)])
    row = row.astype(np.int64)
    col = col.astype(np.int64)

    core = col // OWN
    loc = col - core * OWN
    win = loc // P                         # dest window 0..97
    off = loc - win * P                    # dest offset in window
    gsrc = (row // OWN) * R + (row % OWN)  # padded global source row
    chk = gsrc // CHUNK
    lsrc = gsrc - chk * CHUNK              # chunk-local source row < 25088

    key = (core * NTILES + win) * NCHUNK + chk
    cnt = np.bincount(key, minlength=NCORES * NTILES * NCHUNK).reshape(
        NCORES, NTILES, NCHUNK)
    bwc = np.maximum(1, -(-cnt.max(axis=0) // P))      # [NTILES, NCHUNK]

    # slot order: window blocks of WB; per block chunk passes c=0..3
    slot_base = np.zeros((NTILES, NCHUNK), np.int64)
    win_of, chunk_of = [], []
    segs = []                              # (chunk, b0, b1) per gather call
    pos = 0
    for wb0 in range(0, NTILES, WB):
        wins = list(range(wb0, min(wb0 + WB, NTILES)))
        for c in range(NCHUNK):
            b0 = pos
            for w in wins:
                slot_base[w, c] = pos
                n = int(bwc[w, c])
                win_of += [w] * n
                chunk_of += [c] * n
                pos += n
            segs.append((c, b0, pos))
    NB = pos
    win_of = np.asarray(win_of)
    chunk_of = np.asarray(chunk_of)
    first = np.zeros(NB, bool)
    first[slot_base[:, 0]] = True
    last = np.zeros(NB, bool)
    last[slot_base[:, NCHUNK - 1] + bwc[:, NCHUNK - 1] - 1] = True

    src16 = np.zeros((NCORES, P, NB), np.int16)
    srcg = np.zeros((NCORES, P, NB), np.int32)   # emulator only
    offb = np.full((NCORES, P, NB), 255.0, np.float32)

    order = np.argsort(key, kind="stable")
    key_s = key[order]
    cg = key_s // (NTILES * NCHUNK)
    wg = (key_s // NCHUNK) % NTILES
    chg = key_s % NCHUNK
    grp_start = np.concatenate([[0], np.cumsum(cnt.ravel())[:-1]])
    j = np.arange(key_s.size) - grp_start[key_s]
    b = slot_base[wg, chg] + j // P
    p = j % P
    src16[cg, p, b] = lsrc[order].astype(np.int16)
    srcg[cg, p, b] = gsrc[order].astype(np.int32)
    offb[cg, p, b] = off[order].astype(np.float32)

    deg = np.bincount(col, minlength=N_NODES).astype(np.float32)
    dinv = 1.0 / np.sqrt(deg)

    maxsegb = max(b1 - b0 for _, b0, b1 in segs)
    return dict(NB=NB, win_of=win_of, chunk_of=chunk_of, first=first,
                last=last, segs=segs, maxsegb=maxsegb,
                src16=src16, srcg=srcg, offb=offb, dinv=dinv)


def build_core_inputs(inputs, sched):
    """Per-core named arrays (consumed by emulate + pack_device_maps)."""
    x = np.asarray(inputs["x"], np.float32)
    batch = np.asarray(inputs["batch"], np.int32)
    dinv = sched["dinv"]

    common = {
        "W_in": np.asarray(inputs["W_in"], np.float32),
        "W_mid": np.asarray(inputs["W_mid"], np.float32),
        "W_out": np.asarray(inputs["W_out"], np.float32),
        "W1": np.asarray(inputs["W1"], np.float32),
        "W2": np.asarray(inputs["W2"], np.float32),
        "identity": np.eye(P, dtype=np.float32),
        "iota_bf": np.tile(np.arange(P, dtype=np.float32), (P, 1)).astype(BF),
        "iota_f32": np.tile(np.arange(P, dtype=np.float32), (P, 1)),
        "ones_col": np.ones((P, 1), np.float32),
        "epsb": np.full((P, 1), EPS, np.float32),
        "b2_bc": np.full((P, 1), float(np.asarray(inputs["b2"])[0]), np.float32),
        "b1_bc": np.tile(np.asarray(inputs["b1"], np.float32), (P, 1)),
    }
    for nm in ("in", "mid", "out"):
        common[f"b_{nm}_bc"] = np.tile(np.asarray(inputs[f"b_{nm}"], np.float32),
                                       (P, 1))
        common[f"g_{nm}_bc"] = np.tile(np.asarray(inputs[f"g_{nm}"], np.float32),
                                       (P, 1))
        common[f"be_{nm}_bc"] = np.tile(np.asarray(inputs[f"be_{nm}"], np.float32),
                                        (P, 1))

    maps = []
    for k in range(NCORES):
        lo, hi = k * OWN, (k + 1) * OWN
        xs = np.zeros((R, HID), np.float32)
        xs[:OWN] = x[lo:hi]
        tmp = np.zeros(R, np.float32)
        tmp[:OWN] = dinv[lo:hi]
        dv = np.ascontiguousarray(tmp.reshape(NTILES, P).T)
        tmp = np.full(R, 255.0, np.float32)
        tmp[:OWN] = batch[lo:hi].astype(np.float32)
        bb = np.ascontiguousarray(tmp.reshape(NTILES, P).T)
        m = dict(common)
        m.update({
            "xs": xs,
            "src16": sched["src16"][k],
            "srcg": sched["srcg"][k],
            "offb": sched["offb"][k].astype(BF),
            "dinvs": dv,
            "batchb": bb,
        })
        maps.append(m)
    return maps


def pack_device_maps(maps, sched):
    """Pack named arrays into device in_maps."""
    NB = sched["NB"]
    dev = []
    for m in maps:
        cb = np.zeros((P, CB_COLS), np.float32)
        for n, w in _CB_LAYOUT:
            o = CB_OFF[n]
            a = np.asarray(m[n], np.float32)
            if n == "W2":
                cb[:FC_DIM, o:o + 1] = a
            else:
                cb[:, o:o + w] = a
        # idx16: per segment, wrap idxs (j = batch*128 + p) into 16
        # partitions, col j//16; replicate to all 8 gpsimd core groups.
        idx16 = np.zeros((P, NB * 8), np.int16)
        for c, b0, b1 in sched["segs"]:
            jarr = m["src16"][:, b0:b1].T.reshape(-1)        # [n] batch-major
            blk = jarr.reshape(-1, 16).T                     # [16, n/16]
            idx16[:, b0 * 8:b1 * 8] = np.tile(blk, (8, 1))
        dev.append({
            "xs": m["xs"].astype(BF),
            "srcidx": idx16,
            "offb": m["offb"],
            "iota_bf": m["iota_bf"],
            "cblob": cb,
        })
    return dev


# ------------------------------------------------------------ numpy emulator
def emulate(inputs, sched, maps):
    """Emulation of the device numerics (bf16 activations/weights)."""
    NB = sched["NB"]
    win_of = sched["win_of"]
    layers = (["in"] + ["mid"] * 6 + ["out"])

    xs = [m["xs"].astype(BF) for m in maps]
    for li, nm in enumerate(layers):
        W = maps[0][f"W_{nm}"].astype(BF).astype(np.float32)
        b = maps[0][f"b_{nm}_bc"][0]
        g = maps[0][f"g_{nm}_bc"][0]
        be = maps[0][f"be_{nm}_bc"][0]
        hp = []
        for k in range(NCORES):
            h = xs[k].astype(np.float32) @ W
            dv = maps[k]["dinvs"].T.reshape(-1, 1)
            hp.append((h * dv).astype(BF))
        h_full = np.concatenate(hp, 0).astype(np.float32)
        nxt = []
        for k in range(NCORES):
            m = maps[k]
            out = np.zeros((R, HID), np.float32)
            for bidx in range(NB):
                w = win_of[bidx]
                G = h_full[m["srcg"][:, bidx]]
                offv = m["offb"][:, bidx].astype(np.float32)
                Pm = (offv[:, None] == np.arange(P)).astype(np.float32)
                out[w * P:(w + 1) * P] += Pm.T @ G
            dv = m["dinvs"].T.reshape(-1, 1)
            z = out * dv + b
            mu = z.mean(1, keepdims=True)
            var = ((z - mu) ** 2).mean(1, keepdims=True)
            y = (z - mu) / np.sqrt(var + EPS) * g + be
            y = np.maximum(y, 0.0).astype(BF)
            if nm == "mid" and li % 2 == 0:
                y = (y.astype(np.float32) + xs[k].astype(np.float32)).astype(BF)
            nxt.append(y)
        xs = nxt

    sums = np.zeros((NUM_GRAPHS, HID + 1), np.float32)
    for k in range(NCORES):
        bb = maps[k]["batchb"].astype(np.float32).T.reshape(-1)
        valid = bb < NUM_GRAPHS
        idx = bb[valid].astype(np.int64)
        np.add.at(sums[:, :HID], idx, xs[k].astype(np.float32)[valid])
        np.add.at(sums[:, HID], idx, 1.0)
    pooled = sums[:, :HID] / np.maximum(sums[:, HID:], 1.0)
    z = np.maximum(pooled @ maps[0]["W1"] + maps[0]["b1_bc"][0], 0.0)
    o = z @ maps[0]["W2"] + maps[0]["b2_bc"][0, 0]
    return 1.0 / (1.0 + np.exp(-o))


# ------------------------------------------------------------- bass program
def build_nc(sched, taps=()):
    import concourse.bass as bass
    import concourse.bacc as bacc
    import concourse.tile as tile
    from concourse import mybir

    NB = sched["NB"]
    win_of, first, last = sched["win_of"], sched["first"], sched["last"]
    segs, maxsegb = sched["segs"], sched["maxsegb"]
    f32 = mybir.dt.float32
    bf16 = mybir.dt.bfloat16
    i16 = mybir.dt.int16
    AF = mybir.ActivationFunctionType
    OP = mybir.AluOpType

    nc = bacc.Bacc("TRN2", target_bir_lowering=False, debug=False,
                   num_devices=NCORES)

    xs_d = nc.dram_tensor("xs", [R, HID], bf16, kind="ExternalInput")
    src_d = nc.dram_tensor("srcidx", [P, NB * 8], i16, kind="ExternalInput")
    off_d = nc.dram_tensor("offb", [P, NB], bf16, kind="ExternalInput")
    cb_d = nc.dram_tensor("cblob", [P, CB_COLS], f32, kind="ExternalInput")
    iobf_d = nc.dram_tensor("iota_bf", [P, P], bf16, kind="ExternalInput")
    out_d = nc.dram_tensor("out", [NUM_GRAPHS, 1], f32, kind="ExternalOutput")
    tap_d = {}
    for tn, shp, dt in [("hfull0", [GPAD, HID], bf16),
                        ("x1", [P, NTILES, P], bf16),
                        ("G0", [P, sched["maxsegb"] * P], bf16),
                        ("P0", [P, sched["maxsegb"] * P], bf16)]:
        if tn in taps:
            tap_d[tn] = nc.dram_tensor(tn, shp, dt, kind="ExternalOutput")

    layers = (["in"] + ["mid"] * 6 + ["out"])

    with tile.TileContext(nc) as tc:
        with (
            tc.tile_pool(name="singles", bufs=1) as singles,
            tc.tile_pool(name="xab", bufs=1) as xab,
            tc.tile_pool(name="sbA", bufs=3) as sbA,
            tc.tile_pool(name="hppool", bufs=3) as hppool,
            tc.tile_pool(name="ipool", bufs=4) as ipool,
            tc.tile_pool(name="gpool", bufs=3) as gpool,
            tc.tile_pool(name="ppool", bufs=3) as ppool,
            tc.tile_pool(name="zpool", bufs=3) as zpool,
            tc.tile_pool(name="spool", bufs=4) as spool,
            tc.tile_pool(name="psA", bufs=3, space="PSUM") as psA,
            tc.tile_pool(name="psW", bufs=4, space="PSUM") as psW,
            tc.tile_pool(name="psP", bufs=1, space="PSUM") as psP,
            tc.tile_pool(name="dram", bufs=2, space="DRAM") as dram,
            tc.tile_pool(name="dram1", bufs=1, space="DRAM") as dram1,
        ):
            # ---- constants
            cblob = singles.tile([P, CB_COLS], f32, name="cblob")
            nc.sync.dma_start(cblob[:], cb_d[:, :])
            consts = {}
            for name, o, w in _cb_slices():
                consts[name] = cblob[:, o:o + w]
            consts["W2"] = cblob[0:FC_DIM, CB_OFF["W2"]:CB_OFF["W2"] + 1]
            iota_sb = singles.tile([P, P], bf16, name="iota_sb")
            nc.sync.dma_start(iota_sb[:], iobf_d[:, :])
            off_sb = singles.tile([P, NB], bf16, name="off_sb")
            nc.sync.dma_start(off_sb[:], off_d[:, :])
            dinv_sb = consts["dinvs"]
            bat_sb = consts["batchb"]

            # bf16 copies of matmul weights / identity
            wbf = {}
            for nm in ("in", "mid", "out"):
                t = singles.tile([P, P], bf16, name=f"Wbf_{nm}")
                nc.vector.tensor_copy(t[:], consts[f"W_{nm}"])
                wbf[nm] = t
            ident_bf = singles.tile([P, P], bf16, name="ident_bf")
            nc.vector.tensor_copy(ident_bf[:], consts["identity"])

            xa = xab.tile([P, NTILES, P], bf16, name="xa")
            xb = xab.tile([P, NTILES, P], bf16, name="xb")
            nc.sync.dma_start(
                xa[:], xs_d.rearrange("(t p) f -> p t f", p=P))

            # pooling one-hot strip: Bstrip[p, t, g] = (batch[p,t] == g)
            Bstrip = singles.tile([P, NTILES, P], bf16, name="Bstrip")
            _bat = bat_sb[:, :]
            bat_b = bass.AP(tensor=_bat.tensor, offset=_bat.offset,
                            ap=list(_bat.ap) + [[0, P]])
            _io = consts["iota_f32"][:, :]
            iota_rep = bass.AP(tensor=_io.tensor, offset=_io.offset,
                               ap=[_io.ap[0], [0, NTILES], [1, P]])
            nc.vector.tensor_tensor(out=Bstrip[:], in0=bat_b, in1=iota_rep,
                                    op=OP.is_equal)
            onesbf = singles.tile([P, 1], bf16, name="onesbf")
            nc.vector.tensor_copy(onesbf[:], consts["ones_col"][:])

            def dense_phase(xcur, nm):
                """h' = dinv * (x @ W) per tile -> DRAM h_own (bf16)."""
                h_own = dram.tile([R, HID], bf16, name="h_own")
                W = wbf[nm]
                for t in range(NTILES):
                    xT_ps = psA.tile([P, P], bf16, name="xT_ps", tag="psa")
                    nc.tensor.transpose(xT_ps[:], xcur[:, t, :], ident_bf[:])
                    xT_sb = sbA.tile([P, P], bf16, name="xT_sb")
                    nc.vector.tensor_copy(xT_sb[:], xT_ps[:])
                    h_ps = psA.tile([P, P], f32, name="h_ps", tag="psa")
                    nc.tensor.matmul(h_ps[:], lhsT=xT_sb[:], rhs=W[:],
                                     start=True, stop=True)
                    hp_sb = hppool.tile([P, P], bf16, name="hp_sb")
                    nc.scalar.activation(hp_sb[:], h_ps[:], AF.Copy,
                                         scale=dinv_sb[:, t:t + 1])
                    nc.sync.dma_start(h_own[t * P:(t + 1) * P, :], hp_sb[:])
                return h_own

            def scatter_phase(h_full, hchunks, xcur, xnxt, nm, residual,
                              tap=False):
                b_bc = consts[f"b_{nm}_bc"]
                g_bc = consts[f"g_{nm}_bc"]
                be_bc = consts[f"be_{nm}_bc"]
                accs = {}
                for si, (c, b0, b1) in enumerate(segs):
                    nb = b1 - b0
                    isl = ipool.tile([P, maxsegb * 8], i16, name="isl")
                    nc.sync.dma_start(isl[:, :nb * 8],
                                      src_d[:, b0 * 8:b1 * 8])
                    # gather ucode adds the AP row-offset to the int16 index,
                    # so every source row must be < 32768 from tensor base:
                    # chunk 0 reads h_full directly, chunks 1-3 read copies.
                    src_ap = (h_full[0:CHUNK, :] if c == 0
                              else hchunks[c - 1][:, :])
                    Gt = gpool.tile([P, maxsegb * P], bf16, name="Gt")
                    nc.gpsimd.dma_gather(
                        out_ap=Gt[:, :nb * P].rearrange(
                            "p (b f) -> p b f", b=nb),
                        in_ap=src_ap,
                        idxs_ap=isl[:, :nb * 8],
                        num_idxs=nb * P,
                        num_idxs_reg=nb * P,
                        elem_size=P,
                        single_packet=False,
                    )
                    if tap and si == 0 and "G0" in tap_d:
                        nc.sync.dma_start(tap_d["G0"][:, :nb * P],
                                          Gt[:, :nb * P])
                    Pt = ppool.tile([P, maxsegb * P], bf16, name="Pt")
                    o = off_sb[:, b0:b1]
                    off_b = bass.AP(tensor=o.tensor, offset=o.offset,
                                    ap=list(o.ap) + [[0, P]])
                    i0 = iota_sb[:, :]
                    iota_b = bass.AP(tensor=i0.tensor, offset=i0.offset,
                                     ap=[i0.ap[0], [0, nb], [1, P]])
                    nc.vector.tensor_tensor(
                        out=Pt[:, :nb * P].rearrange("p (b f) -> p b f", b=nb),
                        in0=off_b, in1=iota_b, op=OP.is_equal)
                    if tap and si == 0 and "P0" in tap_d:
                        nc.sync.dma_start(tap_d["P0"][:, :nb * P],
                                          Pt[:, :nb * P])
                    for bi in range(b0, b1):
                        w = int(win_of[bi])
                        s = bi - b0
                        if first[bi]:
                            accs[w] = psW.tile([P, P], f32, name="acc")
                        acc = accs[w]
                        nc.tensor.matmul(
                            acc[:], lhsT=Pt[:, s * P:(s + 1) * P],
                            rhs=Gt[:, s * P:(s + 1) * P],
                            start=bool(first[bi]), stop=bool(last[bi]))
                        if last[bi]:
                            del accs[w]
                            z = zpool.tile([P, P], f32, name="z")
                            nc.scalar.activation(z[:], acc[:], AF.Copy,
                                                 scale=dinv_sb[:, w:w + 1])
                            nc.vector.tensor_add(z[:], z[:], b_bc[:])
                            stats = spool.tile([P, 6], f32, name="stats")
                            nc.vector.bn_stats(stats[:], z[:])
                            mv = spool.tile([P, 2], f32, name="mv")
                            nc.vector.bn_aggr(mv[:], stats[:])
                            sd = spool.tile([P, 1], f32, name="sd")
                            nc.scalar.activation(sd[:], mv[:, 1:2], AF.Sqrt,
                                                 bias=consts["epsb"][:])
                            rstd = spool.tile([P, 1], f32, name="rstd")
                            nc.vector.reciprocal(rstd[:], sd[:])
                            nc.vector.tensor_scalar(
                                out=z[:], in0=z[:], scalar1=mv[:, 0:1],
                                scalar2=rstd[:], op0=OP.subtract, op1=OP.mult)
                            nc.vector.tensor_mul(z[:], z[:], g_bc[:])
                            nc.gpsimd.tensor_add(z[:], z[:], be_bc[:])
                            if residual:
                                y = zpool.tile([P, P], bf16, name="y")
                                nc.scalar.activation(y[:], z[:], AF.Relu)
                                nc.vector.tensor_add(
                                    xnxt[:, w, :], y[:], xcur[:, w, :])
                            else:
                                nc.scalar.activation(xnxt[:, w, :], z[:],
                                                     AF.Relu)

            cur, nxt = xa, xb
            for li, nm in enumerate(layers):
                h_own = dense_phase(cur, nm)
                h_full = dram.tile([GPAD, HID], bf16, addr_space="Shared",
                                   name="h_full")
                nc.gpsimd.collective_compute(
                    "AllGather", OP.bypass,
                    replica_groups=[list(range(NCORES))],
                    ins=[h_own[:, :].opt()], outs=[h_full[:, :].opt()])
                if li == 0 and "hfull0" in tap_d:
                    nc.sync.dma_start(tap_d["hfull0"][:, :], h_full[:, :])
                hchunks = []
                for c in range(1, NCHUNK):
                    hc = dram.tile([CHUNK, HID], bf16, name="hc",
                                   tag=f"hc{c}")
                    nc.sync.dma_start(hc[:, :],
                                      h_full[c * CHUNK:(c + 1) * CHUNK, :])
                    hchunks.append(hc)
                residual = (nm == "mid" and li % 2 == 0)
                scatter_phase(h_full, hchunks, cur, nxt, nm, residual,
                              tap=(li == 0))
                if li == 0 and "x1" in tap_d:
                    nc.sync.dma_start(tap_d["x1"][:, :, :], nxt[:])
                cur, nxt = nxt, cur

            # ---- pooling
            pool_ps = psP.tile([P, HID + 1], f32, name="pool_ps")
            for t in range(NTILES):
                nc.tensor.matmul(pool_ps[:, :HID], lhsT=Bstrip[:, t, :],
                                 rhs=cur[:, t, :],
                                 start=(t == 0), stop=(t == NTILES - 1),
                                 skip_group_check=True)
                nc.tensor.matmul(pool_ps[:, HID:HID + 1], lhsT=Bstrip[:, t, :],
                                 rhs=onesbf[:],
                                 start=(t == 0), stop=(t == NTILES - 1),
                                 skip_group_check=True)
            pool_sb = zpool.tile([P, HID + 1], f32, name="pool_sb")
            nc.vector.tensor_copy(pool_sb[:], pool_ps[:])
            pool_in = dram1.tile([P, HID + 1], f32, name="pool_in")
            pool_out = dram1.tile([P, HID + 1], f32, addr_space="Shared",
                                  name="pool_out")
            nc.sync.dma_start(pool_in[:, :], pool_sb[:])
            nc.gpsimd.collective_compute(
                "AllReduce", OP.add, replica_groups=[list(range(NCORES))],
                ins=[pool_in[:, :].opt()], outs=[pool_out[:, :].opt()])
            pooled = zpool.tile([P, HID + 1], f32, name="pooled")
            nc.sync.dma_start(pooled[:], pool_out[:, :])

            cnt = spool.tile([P, 1], f32, name="cnt")
            nc.vector.tensor_copy(cnt[:], pooled[:, HID:HID + 1])
            nc.vector.tensor_scalar_max(out=cnt[:], in0=cnt[:], scalar1=1.0)
            crec = spool.tile([P, 1], f32, name="crec")
            nc.vector.reciprocal(crec[:], cnt[:])
            pm = zpool.tile([P, HID], f32, name="pm")
            nc.vector.tensor_scalar_mul(out=pm[:], in0=pooled[:, :HID],
                                        scalar1=crec[:])
            pmT_ps = psA.tile([P, P], f32, name="pmT_ps", tag="psa")
            nc.tensor.transpose(pmT_ps[:], pm[:], consts["identity"])
            pmT = sbA.tile([P, P], f32, name="pmT")
            nc.vector.tensor_copy(pmT[:], pmT_ps[:])
            z1_ps = psA.tile([P, FC_DIM], f32, name="z1_ps", tag="psa")
            nc.tensor.matmul(z1_ps[:], lhsT=pmT[:], rhs=consts["W1"],
                             start=True, stop=True)
            z1 = zpool.tile([P, FC_DIM], f32, name="z1")
            nc.vector.tensor_add(z1[:], z1_ps[:], consts["b1_bc"])
            nc.scalar.activation(z1[:], z1[:], AF.Relu)
            z1T_ps = psA.tile([FC_DIM, P], f32, name="z1T_ps", tag="psa")
            nc.tensor.transpose(z1T_ps[:], z1[:], consts["identity"])
            z1T = sbA.tile([FC_DIM, P], f32, name="z1T")
            nc.vector.tensor_copy(z1T[:], z1T_ps[:])
            o_ps = psA.tile([P, 1], f32, name="o_ps", tag="psa")
            nc.tensor.matmul(o_ps[:], lhsT=z1T[:], rhs=consts["W2"],
                             start=True, stop=True)
            o_sb = spool.tile([P, 1], f32, name="o_sb")
            nc.scalar.activation(o_sb[:], o_ps[:], AF.Sigmoid,
                                 bias=consts["b2_bc"][:])
            nc.sync.dma_start(out_d[:, :], o_sb[:])

    nc.compile()
    return nc


# ----------------------------------------------------------------- entry
_CACHE = {}


def kernel(**inputs):
    from concourse import bass_utils

    edge_index = np.asarray(inputs["edge_index"])
    sched = build_schedule(edge_index)
    maps = build_core_inputs(inputs, sched)

    key = sched["NB"]
    if key not in _CACHE:
        _CACHE[key] = build_nc(sched)
    nc = _CACHE[key]

    res = bass_utils.run_bass_kernel_spmd(
        nc, pack_device_maps(maps, sched), core_ids=list(range(NCORES)))
    return np.asarray(res.results[0]["out"], np.float32)



# revision 45
# speedup vs baseline: 13.0613x; 13.0613x over previous
"""Trainium2 Bass kernel for an 8-layer GCN (MemoryEfficientGNN).

Strategy (8 NeuronCores, SPMD single program):
  - Nodes sharded across cores: core k owns real nodes [12500k, 12500(k+1)),
    padded to 12544 rows (98 tiles/windows of 128).
  - GCN norm factorizes: out[v] = dinv[v] * sum_{(u,v)} dinv[u]*h[u], so the
    per-edge weights are one-hot.
  - Per layer: h' = dinv*(x_own @ W) on PE (bf16), AllGathered in 7 SLICES
    (14 tiles each): slice s -> hfull_s [14336,128] bf16 (int16-indexable),
    fired as soon as its tiles finish so collectives overlap the scatter.
  - Scatter: per 128-edge batch, dma_gather source rows from hfull_s
    (4 SWDGE queues round-robin; queue drain is the throughput limit) and
    one-hot matmul P^T.T @ G accumulated in PSUM per 128-dest window
    (window blocks of 4 = 4 PSUM banks, slice passes s=0..6 inside).
  - Post per window: dinv scale + bias + LayerNorm + affine + ReLU
    (+ residual on alternate mid layers) on DVE/ACT. The NEXT layer's dense
    tile for that window is emitted inline (software pipeline), so dense
    PE work and the next layer's AllGather slices hide under the current
    layer's gather stream.
  - Final: per-graph mean pool via one-hot(batch-id) matmuls, AllReduce of
    [128 graphs, 128+1] partials, replicated MLP + sigmoid.

The batch schedule is static and identical on all cores (SPMD); per-core
data (indices, offsets, x rows) differ.
"""

import sys

sys.path.insert(0, "/opt/trn_rl_repo")

import numpy as np
import ml_dtypes

N_NODES = 100000
N_EDGES = 1600000
HID = 128
FC_DIM = 64
NUM_GRAPHS = 128
EPS = 1e-5
NCORES = 8
OWN = N_NODES // NCORES          # 12500 real nodes per core
NTILES = (OWN + 127) // 128      # 98 windows per core
R = NTILES * 128                 # 12544 padded rows per core
GPAD = R * NCORES                # 100352 padded global rows
P = 128
NSLICE = 7                       # AllGather slices per layer
TSL = NTILES // NSLICE           # 14 tiles per slice
SLOC = TSL * P                   # 1792 rows per slice per core
SROWS = SLOC * NCORES            # 14336 rows per hfull slice (int16-safe)
WB = 4                           # windows per PSUM block

BF = ml_dtypes.bfloat16

# fp32 constant-blob layout: one DMA covers every fp32 constant so any
# first-reader sees a single DMA semaphore lane (walrus caps per-inst waits).
_CB_LAYOUT = [
    ("W_in", P), ("W_mid", P), ("W_out", P), ("identity", P), ("iota_f32", P),
    ("b_in_bc", P), ("g_in_bc", P), ("be_in_bc", P),
    ("b_mid_bc", P), ("g_mid_bc", P), ("be_mid_bc", P),
    ("b_out_bc", P), ("g_out_bc", P), ("be_out_bc", P),
    ("b1_bc", FC_DIM), ("W1", FC_DIM), ("W2", 1),
    ("epsb", 1), ("ones_col", 1), ("b2_bc", 1),
    ("dinvs", NTILES), ("batchb", NTILES),
]
CB_OFF = {}
_o = 0
for _n, _w in _CB_LAYOUT:
    CB_OFF[_n] = _o
    _o += _w
CB_COLS = _o


def _cb_slices():
    return [(n, CB_OFF[n], w) for n, w in _CB_LAYOUT if n != "W2"]


# ---------------------------------------------------------------- host prep
def build_schedule(edge_index):
    """Static batch schedule + per-core index/offset arrays.

    Gather chunks are keyed by AllGather SLICE: slice s gathers tile block
    [s*TSL, (s+1)*TSL) of every core into hfull_s = [SROWS, HID]; an edge
    with source row (ksrc, locsrc) lives in slice locsrc // SLOC at local
    row ksrc*SLOC + locsrc % SLOC (< 14336, int16-safe).
    """
    # self-loops contribute h'own[v] to window v//P at offset v%P — handled
    # in-kernel by an identity matmul over the SBUF-resident hp tile, so
    # they are excluded from the gather schedule. Degrees still count them.
    row = edge_index[0].astype(np.int64)
    col = edge_index[1].astype(np.int64)

    core = col // OWN
    loc = col - core * OWN
    win = loc // P                         # dest window 0..97
    off = loc - win * P                    # dest offset in window
    ksrc = row // OWN
    locsrc = row - ksrc * OWN
    chk = locsrc // SLOC                   # AllGather slice 0..6
    lsrc = ksrc * SLOC + (locsrc - chk * SLOC)   # row within hfull_s
    gsrc = ksrc * R + locsrc               # padded global row (emulator)

    key = (core * NTILES + win) * NSLICE + chk
    cnt = np.bincount(key, minlength=NCORES * NTILES * NSLICE).reshape(
        NCORES, NTILES, NSLICE)
    bwc = np.maximum(1, -(-cnt.max(axis=0) // P))      # [NTILES, NSLICE]

    # slot order: window blocks of WB; per block slice passes s=0..6
    slot_base = np.zeros((NTILES, NSLICE), np.int64)
    win_of, chunk_of = [], []
    segs = []                              # (slice, b0, b1) per gather call
    pos = 0
    for wb0 in range(0, NTILES, WB):
        wins = list(range(wb0, min(wb0 + WB, NTILES)))
        for c in range(NSLICE):
            b0 = pos
            for w in wins:
                slot_base[w, c] = pos
                n = int(bwc[w, c])
                win_of += [w] * n
                chunk_of += [c] * n
                pos += n
            segs.append((c, b0, pos))
    NB = pos
    win_of = np.asarray(win_of)
    chunk_of = np.asarray(chunk_of)
    first = np.zeros(NB, bool)
    first[slot_base[:, 0]] = True
    last = np.zeros(NB, bool)
    last[slot_base[:, NSLICE - 1] + bwc[:, NSLICE - 1] - 1] = True

    src16 = np.zeros((NCORES, P, NB), np.int16)
    srcg = np.zeros((NCORES, P, NB), np.int32)   # emulator only
    offb = np.full((NCORES, P, NB), 255.0, np.float32)

    order = np.argsort(key, kind="stable")
    key_s = key[order]
    cg = key_s // (NTILES * NSLICE)
    wg = (key_s // NSLICE) % NTILES
    chg = key_s % NSLICE
    grp_start = np.concatenate([[0], np.cumsum(cnt.ravel())[:-1]])
    j = np.arange(key_s.size) - grp_start[key_s]
    b = slot_base[wg, chg] + j // P
    p = j % P
    src16[cg, p, b] = lsrc[order].astype(np.int16)
    srcg[cg, p, b] = gsrc[order].astype(np.int32)
    offb[cg, p, b] = off[order].astype(np.float32)

    deg = 1.0 + np.bincount(col, minlength=N_NODES).astype(np.float32)
    dinv = 1.0 / np.sqrt(deg)

    maxsegb = max(b1 - b0 for _, b0, b1 in segs)
    return dict(NB=NB, win_of=win_of, chunk_of=chunk_of, first=first,
                last=last, segs=segs, maxsegb=maxsegb,
                src16=src16, srcg=srcg, offb=offb, dinv=dinv)


def build_core_inputs(inputs, sched):
    """Per-core named arrays (consumed by emulate + pack_device_maps)."""
    x = np.asarray(inputs["x"], np.float32)
    batch = np.asarray(inputs["batch"], np.int32)
    dinv = sched["dinv"]

    common = {
        "W_in": np.asarray(inputs["W_in"], np.float32),
        "W_mid": np.asarray(inputs["W_mid"], np.float32),
        "W_out": np.asarray(inputs["W_out"], np.float32),
        "W1": np.asarray(inputs["W1"], np.float32),
        "W2": np.asarray(inputs["W2"], np.float32),
        "identity": np.eye(P, dtype=np.float32),
        "iota_bf": np.tile(np.arange(P, dtype=np.float32), (P, 1)).astype(BF),
        "iota_f32": np.tile(np.arange(P, dtype=np.float32), (P, 1)),
        "ones_col": np.ones((P, 1), np.float32),
        "epsb": np.full((P, 1), EPS, np.float32),
        "b2_bc": np.full((P, 1), float(np.asarray(inputs["b2"])[0]), np.float32),
        "b1_bc": np.tile(np.asarray(inputs["b1"], np.float32), (P, 1)),
    }
    for nm in ("in", "mid", "out"):
        common[f"b_{nm}_bc"] = np.tile(np.asarray(inputs[f"b_{nm}"], np.float32),
                                       (P, 1))
        common[f"g_{nm}_bc"] = np.tile(np.asarray(inputs[f"g_{nm}"], np.float32),
                                       (P, 1))
        common[f"be_{nm}_bc"] = np.tile(np.asarray(inputs[f"be_{nm}"], np.float32),
                                        (P, 1))

    maps = []
    for k in range(NCORES):
        lo, hi = k * OWN, (k + 1) * OWN
        xs = np.zeros((R, HID), np.float32)
        xs[:OWN] = x[lo:hi]
        tmp = np.zeros(R, np.float32)
        tmp[:OWN] = dinv[lo:hi]
        dv = np.ascontiguousarray(tmp.reshape(NTILES, P).T)
        tmp = np.full(R, 255.0, np.float32)
        tmp[:OWN] = batch[lo:hi].astype(np.float32)
        bb = np.ascontiguousarray(tmp.reshape(NTILES, P).T)
        m = dict(common)
        m.update({
            "xs": xs,
            "src16": sched["src16"][k],
            "srcg": sched["srcg"][k],
            "offb": sched["offb"][k].astype(BF),
            "dinvs": dv,
            "batchb": bb,
        })
        maps.append(m)
    return maps


def pack_device_maps(maps, sched):
    """Pack named arrays into device in_maps."""
    NB = sched["NB"]
    dev = []
    for m in maps:
        cb = np.zeros((P, CB_COLS), np.float32)
        for n, w in _CB_LAYOUT:
            o = CB_OFF[n]
            a = np.asarray(m[n], np.float32)
            if n == "W2":
                cb[:FC_DIM, o:o + 1] = a
            else:
                cb[:, o:o + w] = a
        # idx16: per segment, wrap idxs (j = batch*128 + p) into 16
        # partitions, col j//16; replicate to all 8 gpsimd core groups.
        idx16 = np.zeros((P, NB * 8), np.int16)
        for c, b0, b1 in sched["segs"]:
            jarr = m["src16"][:, b0:b1].T.reshape(-1)        # [n] batch-major
            blk = jarr.reshape(-1, 16).T                     # [16, n/16]
            idx16[:, b0 * 8:b1 * 8] = np.tile(blk, (8, 1))
        dev.append({
            "xs": m["xs"].astype(BF),
            "srcidx": idx16,
            "offb": m["offb"],
            "iota_bf": m["iota_bf"],
            "cblob": cb,
        })
    return dev


# ------------------------------------------------------------ numpy emulator
def emulate(inputs, sched, maps):
    """Emulation of the device numerics (bf16 activations/weights)."""
    NB = sched["NB"]
    win_of = sched["win_of"]
    layers = (["in"] + ["mid"] * 6 + ["out"])

    xs = [m["xs"].astype(BF) for m in maps]
    for li, nm in enumerate(layers):
        W = maps[0][f"W_{nm}"].astype(BF).astype(np.float32)
        b = maps[0][f"b_{nm}_bc"][0]
        g = maps[0][f"g_{nm}_bc"][0]
        be = maps[0][f"be_{nm}_bc"][0]
        hp = []
        for k in range(NCORES):
            h = xs[k].astype(np.float32) @ W
            dv = maps[k]["dinvs"].T.reshape(-1, 1)
            hp.append((h * dv).astype(BF))
        h_full = np.concatenate(hp, 0).astype(np.float32)
        nxt = []
        for k in range(NCORES):
            m = maps[k]
            out = hp[k].astype(np.float32).copy()  # self-loop identity term
            for bidx in range(NB):
                w = win_of[bidx]
                G = h_full[m["srcg"][:, bidx]]
                offv = m["offb"][:, bidx].astype(np.float32)
                Pm = (offv[:, None] == np.arange(P)).astype(np.float32)
                out[w * P:(w + 1) * P] += Pm.T @ G
            dv = m["dinvs"].T.reshape(-1, 1)
            z = out * dv + b
            mu = z.mean(1, keepdims=True)
            var = ((z - mu) ** 2).mean(1, keepdims=True)
            y = (z - mu) / np.sqrt(var + EPS) * g + be
            y = np.maximum(y, 0.0).astype(BF)
            if nm == "mid" and li % 2 == 0:
                y = (y.astype(np.float32) + xs[k].astype(np.float32)).astype(BF)
            nxt.append(y)
        xs = nxt

    sums = np.zeros((NUM_GRAPHS, HID + 1), np.float32)
    for k in range(NCORES):
        bb = maps[k]["batchb"].astype(np.float32).T.reshape(-1)
        valid = bb < NUM_GRAPHS
        idx = bb[valid].astype(np.int64)
        np.add.at(sums[:, :HID], idx, xs[k].astype(np.float32)[valid])
        np.add.at(sums[:, HID], idx, 1.0)
    pooled = sums[:, :HID] / np.maximum(sums[:, HID:], 1.0)
    z = np.maximum(pooled @ maps[0]["W1"] + maps[0]["b1_bc"][0], 0.0)
    o = z @ maps[0]["W2"] + maps[0]["b2_bc"][0, 0]
    return 1.0 / (1.0 + np.exp(-o))


# ------------------------------------------------------------- bass program
FULL_PARTS = ("ag", "scatter", "ar")


def build_nc(sched, taps=(), parts=FULL_PARTS, reps=1):
    import concourse.bass as bass
    import concourse.bacc as bacc
    import concourse.tile as tile
    from concourse import mybir

    NB = sched["NB"]
    win_of, first, last = sched["win_of"], sched["first"], sched["last"]
    segs, maxsegb = sched["segs"], sched["maxsegb"]
    f32 = mybir.dt.float32
    bf16 = mybir.dt.bfloat16
    i16 = mybir.dt.int16
    AF = mybir.ActivationFunctionType
    OP = mybir.AluOpType

    nc = bacc.Bacc("TRN2", target_bir_lowering=False, debug=False,
                   num_devices=NCORES, num_swdge_queues=4)

    xs_d = nc.dram_tensor("xs", [R, HID], bf16, kind="ExternalInput")
    src_d = nc.dram_tensor("srcidx", [P, NB * 8], i16, kind="ExternalInput")
    off_d = nc.dram_tensor("offb", [P, NB], bf16, kind="ExternalInput")
    cb_d = nc.dram_tensor("cblob", [P, CB_COLS], f32, kind="ExternalInput")
    iobf_d = nc.dram_tensor("iota_bf", [P, P], bf16, kind="ExternalInput")
    out_d = nc.dram_tensor("out", [NUM_GRAPHS, 1], f32, kind="ExternalOutput")
    tap_d = {}
    for tn, shp, dt in [("hfull0", [GPAD, HID], bf16),
                        ("x1", [P, NTILES, P], bf16),
                        ("G0", [P, sched["maxsegb"] * P], bf16),
                        ("P0", [P, sched["maxsegb"] * P], bf16)]:
        if tn in taps:
            tap_d[tn] = nc.dram_tensor(tn, shp, dt, kind="ExternalOutput")

    layers = (["in"] + ["mid"] * 6 + ["out"])

    with tile.TileContext(nc) as tc:
        with (
            tc.tile_pool(name="singles", bufs=1) as singles,
            tc.tile_pool(name="xab", bufs=1) as xab,
            tc.tile_pool(name="sbA", bufs=3) as sbA,

            tc.tile_pool(name="ipool", bufs=8) as ipool,
            tc.tile_pool(name="gpool", bufs=6) as gpool,
            tc.tile_pool(name="ppool", bufs=6) as ppool,
            tc.tile_pool(name="zpool", bufs=3) as zpool,
            tc.tile_pool(name="spool", bufs=4) as spool,
            tc.tile_pool(name="psA", bufs=3, space="PSUM") as psA,
            tc.tile_pool(name="psW", bufs=4, space="PSUM") as psW,
            tc.tile_pool(name="psP", bufs=1, space="PSUM") as psP,
            tc.tile_pool(name="dram", bufs=2, space="DRAM") as dram,
            tc.tile_pool(name="dramF", bufs=9, space="DRAM") as dramF,
            tc.tile_pool(name="dram1", bufs=1, space="DRAM") as dram1,
        ):
            # ---- constants
            cblob = singles.tile([P, CB_COLS], f32, name="cblob")
            nc.sync.dma_start(cblob[:], cb_d[:, :])
            consts = {}
            for name, o, w in _cb_slices():
                consts[name] = cblob[:, o:o + w]
            consts["W2"] = cblob[0:FC_DIM, CB_OFF["W2"]:CB_OFF["W2"] + 1]
            iota_sb = singles.tile([P, P], bf16, name="iota_sb")
            nc.sync.dma_start(iota_sb[:], iobf_d[:, :])
            off_sb = singles.tile([P, NB], bf16, name="off_sb")
            nc.sync.dma_start(off_sb[:], off_d[:, :])
            dinv_sb = consts["dinvs"]
            bat_sb = consts["batchb"]

            # bf16 copies of matmul weights / identity
            wbf = {}
            for nm in ("in", "mid", "out"):
                t = singles.tile([P, P], bf16, name=f"Wbf_{nm}")
                nc.vector.tensor_copy(t[:], consts[f"W_{nm}"])
                wbf[nm] = t
            ident_bf = singles.tile([P, P], bf16, name="ident_bf")
            nc.vector.tensor_copy(ident_bf[:], consts["identity"])

            xa = xab.tile([P, NTILES, P], bf16, name="xa")
            xb = xab.tile([P, NTILES, P], bf16, name="xb")
            hpA = xab.tile([P, NTILES, P], bf16, name="hpA")
            hpB = xab.tile([P, NTILES, P], bf16, name="hpB")
            nc.sync.dma_start(
                xa[:], xs_d.rearrange("(t p) f -> p t f", p=P))

            # pooling one-hot strip: Bstrip[p, t, g] = (batch[p,t] == g)
            Bstrip = singles.tile([P, NTILES, P], bf16, name="Bstrip")
            _bat = bat_sb[:, :]
            bat_b = bass.AP(tensor=_bat.tensor, offset=_bat.offset,
                            ap=list(_bat.ap) + [[0, P]])
            _io = consts["iota_f32"][:, :]
            iota_rep = bass.AP(tensor=_io.tensor, offset=_io.offset,
                               ap=[_io.ap[0], [0, NTILES], [1, P]])
            nc.vector.tensor_tensor(out=Bstrip[:], in0=bat_b, in1=iota_rep,
                                    op=OP.is_equal)
            onesbf = singles.tile([P, 1], bf16, name="onesbf")
            nc.vector.tensor_copy(onesbf[:], consts["ones_col"][:])

            def dense_phase(xcur, nm, hp):
                """h' = dinv * (x @ W) per tile -> DRAM h_own (bf16); fires
                the slice-s AllGather as soon as its TSL tiles are written.
                Returns the list of NSLICE hfull slice tensors."""
                h_own = dram.tile([R, HID], bf16, name="h_own")
                hfulls = []
                for t in range(NTILES):
                    dense_tile(xcur, nm, t, h_own, hfulls, hp)
                return hfulls

            def dense_tile(xcur, nm, t, h_own, hfulls_out, hp):
                """One dense tile + slice AllGather fire at slice end."""
                W = wbf[nm]
                xT_ps = psA.tile([P, P], bf16, name="xT_ps", tag="psa")
                nc.tensor.transpose(xT_ps[:], xcur[:, t, :], ident_bf[:])
                xT_sb = sbA.tile([P, P], bf16, name="xT_sb")
                nc.vector.tensor_copy(xT_sb[:], xT_ps[:])
                h_ps = psA.tile([P, P], f32, name="h_ps", tag="psa")
                nc.tensor.matmul(h_ps[:], lhsT=xT_sb[:], rhs=W[:],
                                 start=True, stop=True)
                nc.scalar.activation(hp[:, t, :], h_ps[:], AF.Copy,
                                     scale=dinv_sb[:, t:t + 1])
                nc.sync.dma_start(h_own[t * P:(t + 1) * P, :], hp[:, t, :])
                if t % TSL == TSL - 1:
                    s = t // TSL
                    hf = dramF.tile([SROWS, HID], bf16,
                                    addr_space="Shared", name="hfull")
                    if "ag" in parts:
                        nc.gpsimd.collective_compute(
                            "AllGather", OP.bypass,
                            replica_groups=[list(range(NCORES))],
                            ins=[h_own[s * SLOC:(s + 1) * SLOC, :].opt()],
                            outs=[hf[:, :].opt()])
                    hfulls_out.append(hf)

            def scatter_phase(hfulls, xcur, xnxt, nm, residual, hp_cur,
                              hp_nxt, next_nm=None, tap=False):
                b_bc = consts[f"b_{nm}_bc"]
                g_bc = consts[f"g_{nm}_bc"]
                be_bc = consts[f"be_{nm}_bc"]
                accs = {}
                hf_next = []
                h_own_n = (dram.tile([R, HID], bf16, name="h_own")
                           if next_nm is not None else None)
                gonly = "gonly" in parts
                noln = "noln" in parts
                for si, (c, b0, b1) in enumerate(segs):
                    nb = b1 - b0
                    isl = ipool.tile([P, maxsegb * 8], i16, name="isl")
                    nc.sync.dma_start(isl[:, :nb * 8],
                                      src_d[:, b0 * 8:b1 * 8])
                    src_ap = hfulls[c][:, :]
                    Gt = gpool.tile([P, maxsegb * P], bf16, name="Gt")
                    nc.gpsimd.dma_gather(
                        out_ap=Gt[:, :nb * P].rearrange(
                            "p (b f) -> p b f", b=nb),
                        in_ap=src_ap,
                        idxs_ap=isl[:, :nb * 8],
                        num_idxs=nb * P,
                        num_idxs_reg=nb * P,
                        elem_size=P,
                        single_packet=False,
                        queue_num=si % 4,
                    )
                    if tap and si == 0 and "G0" in tap_d:
                        nc.sync.dma_start(tap_d["G0"][:, :nb * P],
                                          Gt[:, :nb * P])
                    if gonly:
                        continue
                    Pt = ppool.tile([P, maxsegb * P], bf16, name="Pt")
                    o = off_sb[:, b0:b1]
                    off_b = bass.AP(tensor=o.tensor, offset=o.offset,
                                    ap=list(o.ap) + [[0, P]])
                    i0 = iota_sb[:, :]
                    iota_b = bass.AP(tensor=i0.tensor, offset=i0.offset,
                                     ap=[i0.ap[0], [0, nb], [1, P]])
                    nc.vector.tensor_tensor(
                        out=Pt[:, :nb * P].rearrange("p (b f) -> p b f", b=nb),
                        in0=off_b, in1=iota_b, op=OP.is_equal)
                    if tap and si == 0 and "P0" in tap_d:
                        nc.sync.dma_start(tap_d["P0"][:, :nb * P],
                                          Pt[:, :nb * P])
                    for bi in range(b0, b1):
                        w = int(win_of[bi])
                        s = bi - b0
                        if first[bi]:
                            # self-loop term: acc starts from h'own[w]
                            # (identity matmul; no gather needed)
                            accs[w] = psW.tile([P, P], f32, name="acc")
                            nc.tensor.matmul(
                                accs[w][:], lhsT=ident_bf[:],
                                rhs=hp_cur[:, w, :], start=True, stop=False)
                        acc = accs[w]
                        nc.tensor.matmul(
                            acc[:], lhsT=Pt[:, s * P:(s + 1) * P],
                            rhs=Gt[:, s * P:(s + 1) * P],
                            start=False, stop=bool(last[bi]))
                        if last[bi]:
                            del accs[w]
                            if noln:
                                nc.scalar.activation(xnxt[:, w, :], acc[:],
                                                     AF.Relu)
                                if next_nm is not None:
                                    dense_tile(xnxt, next_nm, w, h_own_n,
                                               hf_next, hp_nxt)
                                continue
                            z = zpool.tile([P, P], f32, name="z")
                            nc.scalar.activation(z[:], acc[:], AF.Copy,
                                                 scale=dinv_sb[:, w:w + 1])
                            nc.vector.tensor_add(z[:], z[:], b_bc[:])
                            stats = spool.tile([P, 6], f32, name="stats")
                            nc.vector.bn_stats(stats[:], z[:])
                            mv = spool.tile([P, 2], f32, name="mv")
                            nc.vector.bn_aggr(mv[:], stats[:])
                            sd = spool.tile([P, 1], f32, name="sd")
                            nc.scalar.activation(sd[:], mv[:, 1:2], AF.Sqrt,
                                                 bias=consts["epsb"][:])
                            rstd = spool.tile([P, 1], f32, name="rstd")
                            nc.vector.reciprocal(rstd[:], sd[:])
                            nc.vector.tensor_scalar(
                                out=z[:], in0=z[:], scalar1=mv[:, 0:1],
                                scalar2=rstd[:], op0=OP.subtract, op1=OP.mult)
                            nc.vector.tensor_mul(z[:], z[:], g_bc[:])
                            nc.vector.tensor_add(z[:], z[:], be_bc[:])
                            if residual:
                                y = zpool.tile([P, P], bf16, name="y")
                                nc.scalar.activation(y[:], z[:], AF.Relu)
                                nc.vector.tensor_add(
                                    xnxt[:, w, :], y[:], xcur[:, w, :])
                            else:
                                nc.scalar.activation(xnxt[:, w, :], z[:],
                                                     AF.Relu)
                            if next_nm is not None:
                                dense_tile(xnxt, next_nm, w, h_own_n,
                                           hf_next, hp_nxt)
                return hf_next

            cur, nxt = xa, xb
            seq = [(li, nm) for _ in range(reps)
                   for li, nm in enumerate(layers)]
            hps = [hpA, hpB]
            pool_ps = psP.tile([P, HID + 1], f32, name="pool_ps")
            pool_fused = False
            if "scatter" in parts and "gonly" in parts:
                for li, nm in seq:
                    hfulls = dense_phase(cur, nm, hpA)
                    scatter_phase(hfulls, cur, nxt, nm, False, hpA, hpB,
                                  next_nm=None)
            elif "scatter" in parts:
                hfulls = dense_phase(cur, seq[0][1], hpA)
                for i, (li, nm) in enumerate(seq):
                    if li == 0 and "hfull0" in tap_d:
                        for s, hf in enumerate(hfulls):
                            nc.sync.dma_start(
                                tap_d["hfull0"][s * SROWS:(s + 1) * SROWS, :],
                                hf[:, :])
                    residual = (nm == "mid" and li % 2 == 0)
                    next_nm = seq[i + 1][1] if i + 1 < len(seq) else None
                    hfulls = scatter_phase(hfulls, cur, nxt, nm, residual,
                                           hps[i % 2], hps[(i + 1) % 2],
                                           next_nm=next_nm, tap=(li == 0))
                    if li == 0 and "x1" in tap_d:
                        nc.sync.dma_start(tap_d["x1"][:, :, :], nxt[:])
                    cur, nxt = nxt, cur
            else:
                for li, nm in seq:
                    hfulls = dense_phase(cur, nm, hpA)

            # ---- pooling (standalone only when not fused into last scatter)
            if not pool_fused:
                for t in range(NTILES):
                    nc.tensor.matmul(pool_ps[:, :HID], lhsT=Bstrip[:, t, :],
                                     rhs=cur[:, t, :],
                                     start=(t == 0), stop=(t == NTILES - 1),
                                     skip_group_check=True)
                    nc.tensor.matmul(pool_ps[:, HID:HID + 1],
                                     lhsT=Bstrip[:, t, :], rhs=onesbf[:],
                                     start=(t == 0), stop=(t == NTILES - 1),
                                     skip_group_check=True)
            pool_sb = zpool.tile([P, HID + 1], f32, name="pool_sb")
            nc.vector.tensor_copy(pool_sb[:], pool_ps[:])
            pooled = zpool.tile([P, HID + 1], f32, name="pooled")
            if "ar" in parts:
                pool_in = dram1.tile([P, HID + 1], f32, name="pool_in")
                pool_out = dram1.tile([P, HID + 1], f32, addr_space="Shared",
                                      name="pool_out")
                nc.sync.dma_start(pool_in[:, :], pool_sb[:])
                nc.gpsimd.collective_compute(
                    "AllReduce", OP.add, replica_groups=[list(range(NCORES))],
                    ins=[pool_in[:, :].opt()], outs=[pool_out[:, :].opt()])
                nc.sync.dma_start(pooled[:], pool_out[:, :])
            else:
                nc.vector.tensor_copy(pooled[:], pool_sb[:])

            cnt = spool.tile([P, 1], f32, name="cnt")
            nc.vector.tensor_copy(cnt[:], pooled[:, HID:HID + 1])
            nc.vector.tensor_scalar_max(out=cnt[:], in0=cnt[:], scalar1=1.0)
            crec = spool.tile([P, 1], f32, name="crec")
            nc.vector.reciprocal(crec[:], cnt[:])
            pm = zpool.tile([P, HID], f32, name="pm")
            nc.vector.tensor_scalar_mul(out=pm[:], in0=pooled[:, :HID],
                                        scalar1=crec[:])
            pmT_ps = psA.tile([P, P], f32, name="pmT_ps", tag="psa")
            nc.tensor.transpose(pmT_ps[:], pm[:], consts["identity"])
            pmT = sbA.tile([P, P], f32, name="pmT")
            nc.vector.tensor_copy(pmT[:], pmT_ps[:])
            z1_ps = psA.tile([P, FC_DIM], f32, name="z1_ps", tag="psa")
            nc.tensor.matmul(z1_ps[:], lhsT=pmT[:], rhs=consts["W1"],
                             start=True, stop=True)
            z1 = zpool.tile([P, FC_DIM], f32, name="z1")
            nc.vector.tensor_add(z1[:], z1_ps[:], consts["b1_bc"])
            nc.scalar.activation(z1[:], z1[:], AF.Relu)
            z1T_ps = psA.tile([FC_DIM, P], f32, name="z1T_ps", tag="psa")
            nc.tensor.transpose(z1T_ps[:], z1[:], consts["identity"])
            z1T = sbA.tile([FC_DIM, P], f32, name="z1T")
            nc.vector.tensor_copy(z1T[:], z1T_ps[:])
            o_ps = psA.tile([P, 1], f32, name="o_ps", tag="psa")
            nc.tensor.matmul(o_ps[:], lhsT=z1T[:], rhs=consts["W2"],
                             start=True, stop=True)
            o_sb = spool.tile([P, 1], f32, name="o_sb")
            nc.scalar.activation(o_sb[:], o_ps[:], AF.Sigmoid,
                                 bias=consts["b2_bc"][:])
            nc.sync.dma_start(out_d[:, :], o_sb[:])

    nc.compile()
    return nc


# ----------------------------------------------------------------- entry
_CACHE = {}


def kernel(**inputs):
    from concourse import bass_utils

    edge_index = np.asarray(inputs["edge_index"])
    sched = build_schedule(edge_index)
    maps = build_core_inputs(inputs, sched)

    key = sched["NB"]
    if key not in _CACHE:
        _CACHE[key] = build_nc(sched)
    nc = _CACHE[key]

    res = bass_utils.run_bass_kernel_spmd(
        nc, pack_device_maps(maps, sched), core_ids=list(range(NCORES)))
    return np.asarray(res.results[0]["out"], np.float32)

